# revision 18
# baseline (speedup 1.0000x reference)
"""Trainium2 Bass kernel for a 3-layer bidirectional projected-LSTM embedder.

Model (from the reference):
  T=160, B=640, F=40, HID=768, PROJ=256, 3 stacked LSTM-with-projection
  layers per direction (fw, bw).  Per step:
      z = [x_t, h_{t-1}] @ Wk + b            # [B, 4*HID], gate order i,j,f,o
      c = sig(f+1)*c + sig(i)*tanh(j)
      h = (sig(o)*tanh(c)) @ Wp              # [B, PROJ]
  Output = l2norm((concat(fw,bw)[t=0] + concat(fw,bw)[t=T-1]) / 2)  # [B, 512]

Sharding: 4 batch groups x 2 directions = 8 cores, BC=160 per core, one
direction per core.  The bw cores receive the input time-reversed host-side,
so every core runs the SAME program (pure forward scan); the host maps the
two saved end-states back to true time.

Layout: feature-major.  All on-chip tensors are [feature(128-partitions),
batch(160)] tiles; z^T = Wk^T @ [x;h]^T is computed with the bf16 weights as
the 128x128 stationary operand (fast-weight-load keeps LDWEIGHTS hidden
under the 160-column stream) and the bf16 activations streaming (the walrus
verifier requires matching dtypes when either operand is fp32).  Gates come
out gate-major, so gate activations, the c update, and s = sig(o)*tanh(c)
need no transposes, and h^T = Wp^T @ s^T is again feature-major -- the
recurrence closes with zero per-step transposes.  The layer-0 x operand is
zero-padded from K=40 to K=128: matmul cost depends on columns, not K, and
a K<128 stationary in the stream both breaks LDWEIGHTS pipelining and drops
PE array duty enough to trip the HAM clock-throttle every super-step.

The three layers run as a pipelined wavefront (super-step n runs layer l at
step n-l), so each layer's ACT/DVE gate chain hides under the other layers'
matmuls.  Wk column order is permuted host-side to [j|i|f|o] so each gate is
one contiguous 2-PSUM-bank block consumed by a single ACT instruction.
Layer-to-layer h stays in SBUF (no DRAM round-trips).  The final
(t0+tT)/2 + l2-normalize is done on the host in numpy.
"""

import numpy as np

T, B, F = 160, 640, 40
HID, PROJ = 768, 256
NG = 4 * HID          # 3072
NCORES = 8
NGRP = 4              # batch groups
BC = B // NGRP        # 160 per core
NT = NG // 128        # 24 gate n-tiles
GPT = 6               # n-tiles per gate
XCH = 16              # x-input DMA chunk (time steps)

_BUILD_CACHE = {}
DEBUG_DUMP = False

# Wk column permutation: gate order i,j,f,o -> [j | i | f | o]
_WK_PERM = np.concatenate([np.arange(768, 1536), np.arange(0, 768),
                           np.arange(1536, 2304), np.arange(2304, 3072)])


def _build(use_bias, t_steps):
    from contextlib import ExitStack

    import concourse.bass as bass  # noqa: F401
    import concourse.tile as tile
    from concourse import bacc, mybir

    f32 = mybir.dt.float32
    f32r = mybir.dt.float32r
    bf16 = mybir.dt.bfloat16
    AF = mybir.ActivationFunctionType

    nc = bacc.Bacc(None, target_bir_lowering=False)

    xT = nc.declare_dram_parameter("xT", [F, t_steps * BC], bf16, isOutput=False)
    wk_in = []
    wp_in = []
    bs_in = []
    for l in range(3):
        ind = F if l == 0 else PROJ
        wk_in.append(nc.declare_dram_parameter(
            f"Wk{l}", [ind + PROJ, NG], bf16, isOutput=False))
        wp_in.append(nc.declare_dram_parameter(
            f"Wp{l}", [HID, PROJ], bf16, isOutput=False))
        if use_bias:
            bs_in.append(nc.declare_dram_parameter(
                f"bs{l}", [128, NT], f32, isOutput=False))
    # h^T of the top layer after the first and last step: [end, 128, m*BC]
    out_ends = nc.declare_dram_parameter(
        "out_ends", [2, 128, 2 * BC], f32r, isOutput=True)
    if DEBUG_DUMP:
        dbg = nc.declare_dram_parameter(
            "dbg", [10, 128, GPT * BC], f32, isOutput=True)

    # k-tile row spans of Wk per layer: x-part rows then h-part rows
    def kspans(l):
        if l == 0:
            return [(0, F)], [(F, 128), (F + 128, 128)]
        return [(0, 128), (128, 128)], [(256, 128), (384, 128)]

    with tile.TileContext(nc) as tc:
        with ExitStack() as top:
            wpool = top.enter_context(tc.tile_pool(name="w", bufs=1))
            st = top.enter_context(tc.tile_pool(name="st", bufs=1))
            hpool = top.enter_context(tc.tile_pool(name="h", bufs=2))
            xpool = top.enter_context(tc.tile_pool(name="x", bufs=2))
            zpool = top.enter_context(
                tc.tile_pool(name="z", bufs=3, space="PSUM"))
            hps = top.enter_context(
                tc.tile_pool(name="hp", bufs=2, space="PSUM"))

            # ---- x-input buffers first: the first matmul needs the x
            # chunk, so its DMA must not queue behind all the weight DMAs
            h_cur = [None, None, None]   # most recent h^T tile   [128, 2*BC]
            xin = [None, None]           # x chunk double buffer
            xbufs = []                   # pre-zeroed [128, .] chunk buffers
            # x is padded to K=128 with zero rows 40:128 so the layer-0
            # x-part matmul is a normal full-K MM (K=40 stalls the LDW
            # pipeline and trips the HAM throttle every super-step).
            for b in range(2):
                xb = xpool.tile([128, XCH * BC], bf16, name=f"xin{b}",
                                tag=f"xin{b}")
                nc.vector.memset(xb, 0)
                xbufs.append(xb)

            def load_xchunk(ch):
                nch = (t_steps + XCH - 1) // XCH
                if ch >= nch:
                    return
                n = min(XCH, t_steps - ch * XCH)
                xt_sb = xbufs[ch % 2]
                nc.sync.dma_start(
                    out=xt_sb[0:F, 0:n * BC],
                    in_=xT[:, ch * XCH * BC:(ch * XCH + n) * BC])
                xin[ch % 2] = xt_sb

            load_xchunk(0)

            # ---- weights to SBUF (layer order: a layer's tiles are only
            # needed once the wavefront reaches it) ----
            wkx, wkh, wpt, bst = [], [], [], []
            for l in range(3):
                xs, hs = kspans(l)
                xt, ht = [], []
                for ki, (r0, rc) in enumerate(xs + hs):
                    pad = 128 if rc < 128 else rc
                    wt = wpool.tile([pad, NG], bf16, name=f"wk{l}_{ki}",
                                    tag=f"wk{l}_{ki}")
                    if pad != rc:
                        nc.vector.memset(wt, 0)
                    # one transfer per k-tile: chunked DMAs pay ~0.6us queue
                    # overhead each and serialize ahead of the first matmul
                    nc.sync.dma_start(out=wt[0:rc, :],
                                      in_=wk_in[l][r0:r0 + rc, :])
                    (xt if ki < len(xs) else ht).append(wt)
                wkx.append(xt)
                wkh.append(ht)
                pt = []
                for ki in range(6):
                    w = wpool.tile([128, PROJ], bf16, name=f"wp{l}_{ki}",
                                   tag=f"wp{l}_{ki}")
                    nc.sync.dma_start(
                        out=w, in_=wp_in[l][ki * 128:(ki + 1) * 128, :])
                    pt.append(w)
                wpt.append(pt)
                if use_bias:
                    bt = wpool.tile([128, NT], f32, name=f"bs{l}", tag=f"bs{l}")
                    nc.sync.dma_start(out=bt, in_=bs_in[l])
                    bst.append(bt)

            # ---- persistent per-layer state ----
            c_sb = [st.tile([128, GPT * BC], f32, name=f"c{l}", tag=f"c{l}")
                    for l in range(3)]
            gsb = [[st.tile([128, GPT * BC], f32, name=f"g{l}_{g}",
                            tag=f"g{l}_{g}") for g in range(4)]
                   for l in range(3)]
            tc_sb = [st.tile([128, GPT * BC], f32, name=f"tc{l}", tag=f"tc{l}")
                     for l in range(3)]
            tmp_sb = [st.tile([128, GPT * BC], f32, name=f"tm{l}", tag=f"tm{l}")
                      for l in range(3)]
            s_sb = [st.tile([128, GPT * BC], bf16, name=f"s{l}", tag=f"s{l}")
                    for l in range(3)]


            def emit_z_act_dve(l, s):
                # gather rhs k-tiles (f32r activations) + matching weights
                if l == 0:
                    if s % XCH == 0:
                        load_xchunk(s // XCH + 1)
                    xt_sb = xin[(s // XCH) % 2]
                    off = (s % XCH) * BC
                    rhs = [xt_sb[:, off:off + BC]]
                else:
                    hb = h_cur[l - 1]
                    rhs = [hb[:, 0:BC], hb[:, BC:2 * BC]]
                wts = list(wkx[l])
                if s > 0:
                    ho = h_cur[l]
                    rhs += [ho[:, 0:BC], ho[:, BC:2 * BC]]
                    wts += wkh[l]
                nk = len(rhs)

                # z matmuls + gate activation, one gate (6 n-tiles) at a time
                for g in range(4):
                    zg = zpool.tile([128, 1024], f32, name=f"z{l}", tag="zg")
                    for n6 in range(GPT):
                        nt = g * GPT + n6
                        dst = zg[:, (n6 // 3) * 512 + (n6 % 3) * BC:]
                        dst = dst[:, 0:BC]
                        for ki in range(nk):
                            # start=True bulk-clears the whole PSUM bank's
                            # has_written bits, racing with the previous
                            # group's drain -- only the first group per bank
                            # may clear; later groups rely on per-element
                            # overwrite-where-unset.
                            nc.tensor.matmul(
                                dst, wts[ki][:, nt * 128:(nt + 1) * 128],
                                rhs[ki],
                                start=(ki == 0 and n6 % 3 == 0),
                                stop=(ki == nk - 1),
                                skip_group_check=(n6 % 3 != 0))
                    fn = AF.Tanh if g == 0 else AF.Sigmoid
                    gd = gsb[l][g]
                    if use_bias:
                        for n6 in range(GPT):
                            nt = g * GPT + n6
                            src = zg[:, (n6 // 3) * 512 + (n6 % 3) * BC:]
                            nc.scalar.activation(
                                gd[:, n6 * BC:(n6 + 1) * BC], src[:, 0:BC],
                                fn, bias=bst[l][:, nt:nt + 1],
                                scale=1.0)
                    else:
                        bias = 1.0 if g == 2 else 0.0
                        src = zg.rearrange("p (b c) -> p b c", b=2)[:, :, 0:3 * BC]
                        dstv = gd.rearrange("p (b c) -> p b c", b=2)
                        nc.scalar.activation(dstv, src, fn, bias=bias)
                    if DEBUG_DUMP and g == 0 and (l, s) in ((0, 0), (0, 1), (1, 0), (1, 1), (2, 0)):
                        slot = {(0, 0): 0, (0, 1): 1, (1, 0): 2,
                                (1, 1): 3, (2, 0): 4}[(l, s)]
                        zt = st.tile([128, GPT * BC], f32, name=f"dbz{slot}",
                                     tag=f"dbz{slot}")
                        nc.vector.tensor_copy(
                            zt.rearrange("p (b c) -> p b c", b=2),
                            zg.rearrange("p (b c) -> p b c", b=2)[:, :, 0:3 * BC])
                        nc.sync.dma_start(out=dbg[slot], in_=zt)

                # c = sig(f+1)*c + sig(i)*tanh(j);  s = sig(o)*tanh(c)
                gj, gi, gf, go = gsb[l]
                if s == 0:
                    nc.vector.tensor_mul(c_sb[l], gi, gj)
                else:
                    nc.vector.tensor_mul(tmp_sb[l], gi, gj)
                    nc.vector.tensor_mul(c_sb[l], gf, c_sb[l])
                    nc.vector.tensor_add(c_sb[l], c_sb[l], tmp_sb[l])
                nc.scalar.activation(tc_sb[l], c_sb[l], AF.Tanh)
                nc.vector.tensor_mul(s_sb[l], go, tc_sb[l])
                if DEBUG_DUMP and (l, s) in ((0, 0), (1, 0)):
                    slot = 8 if l == 0 else 9
                    st9 = st.tile([128, GPT * BC], f32, name=f"dbs{slot}",
                                  tag=f"dbs{slot}")
                    nc.vector.tensor_copy(st9, s_sb[l])
                    nc.sync.dma_start(out=dbg[slot], in_=st9)

            def emit_wp(l, s):
                hp = hps.tile([128, 2 * BC], f32, name="hps", tag="hps")
                for m in range(2):
                    for ki in range(6):
                        nc.tensor.matmul(
                            hp[:, m * BC:(m + 1) * BC],
                            wpt[l][ki][:, m * 128:(m + 1) * 128],
                            s_sb[l][:, ki * BC:(ki + 1) * BC],
                            start=(ki == 0 and m == 0), stop=(ki == 5),
                            skip_group_check=(m == 1))
                hn = hpool.tile([128, 2 * BC], bf16, name=f"hn{l}", tag=f"hn{l}")
                nc.vector.tensor_copy(hn, hp)
                h_cur[l] = hn
                if DEBUG_DUMP and (l, s) in ((0, 0), (1, 0), (0, 1)):
                    slot = {(0, 0): 5, (1, 0): 6, (0, 1): 7}[(l, s)]
                    ht = st.tile([128, 2 * BC], f32, name=f"dbh{slot}",
                                 tag=f"dbh{slot}")
                    nc.vector.tensor_copy(ht, hp)
                    nc.sync.dma_start(out=dbg[slot][:, 0:2 * BC], in_=ht)
                if l == 2 and (s == 0 or s == t_steps - 1):
                    end = 0 if s == 0 else 1
                    oc = st.tile([128, 2 * BC], f32r, name=f"oc{end}",
                                 tag=f"oc{end}")
                    nc.vector.tensor_copy(oc, hp)
                    nc.sync.dma_start(out=out_ends[end], in_=oc)

            # ---- wavefront: super-step n runs layer l at step n-l; each
            # layer's Wp matmul is emitted one z-block later so the PE never
            # waits on the ACT/DVE gate chain.
            wp_queue = []
            for ss in range(t_steps + 2):
                for l in range(3):
                    s = ss - l
                    if not (0 <= s < t_steps):
                        continue
                    # flush any deferred Wp this z-block depends on
                    need = {(l, s - 1), (l - 1, s)}
                    while any(x in wp_queue for x in need):
                        emit_wp(*wp_queue.pop(0))
                    emit_z_act_dve(l, s)
                    wp_queue.append((l, s))
                    if len(wp_queue) > 1:
                        emit_wp(*wp_queue.pop(0))
            while wp_queue:
                emit_wp(*wp_queue.pop(0))

    nc.finalize()
    return nc


def _get_nc(use_bias, t_steps):
    key = (use_bias, t_steps)
    if key not in _BUILD_CACHE:
        _BUILD_CACHE[key] = _build(use_bias, t_steps)
    return _BUILD_CACHE[key]


def make_in_maps(inputs, t_steps=T):
    """Per-core input dicts.  Cores 0-3: fw, batch groups 0-3.
    Cores 4-7: bw (time-reversed input), batch groups 0-3."""
    import ml_dtypes

    inp = {k: np.asarray(v, dtype=np.float32) for k, v in inputs.items()}
    batch = inp["batch"][:t_steps]

    shared = {}
    for d in ("fw", "bw"):
        for l in range(3):
            shared[d, f"Wk{l}"] = np.ascontiguousarray(
                inp[f"Wk_{d}{l}"][:, _WK_PERM].astype(ml_dtypes.bfloat16))
            shared[d, f"Wp{l}"] = np.ascontiguousarray(
                inp[f"Wp_{d}{l}"].astype(ml_dtypes.bfloat16))
            b = inp[f"b_{d}{l}"][_WK_PERM]
            shared[d, f"bs{l}"] = np.ascontiguousarray(
                b.reshape(NT, 128).T.astype(np.float32))

    use_bias = any(np.any(inp[f"b_{d}{l}"])
                   for d in ("fw", "bw") for l in range(3))
    in_maps = []
    for i in range(NCORES):
        d = "fw" if i < NGRP else "bw"
        g = i % NGRP
        xb = batch[:, g * BC:(g + 1) * BC, :]            # [T, BC, F]
        if d == "bw":
            xb = xb[::-1]
        xT_i = np.ascontiguousarray(
            xb.transpose(2, 0, 1).reshape(F, t_steps * BC)
            .astype(ml_dtypes.bfloat16))
        m = {"xT": xT_i}
        for l in range(3):
            m[f"Wk{l}"] = shared[d, f"Wk{l}"]
            m[f"Wp{l}"] = shared[d, f"Wp{l}"]
            if use_bias:
                m[f"bs{l}"] = shared[d, f"bs{l}"]
        in_maps.append(m)
    return in_maps, use_bias


def assemble(results, t_steps=T):
    """results[i]["out_ends"]: [end, m, 128, BC] -> final [B, 2*PROJ] f32."""
    h = np.zeros((2, 2, B, PROJ), dtype=np.float32)   # [dir, end, B, PROJ]
    for i in range(NCORES):
        di, g = (0, i) if i < NGRP else (1, i - NGRP)
        oe = np.asarray(results[i]["out_ends"], dtype=np.float32)
        # oe: [end, p, m*BC];  h[b, m*128 + p] = oe[end, p, m*BC + b]
        h[di, :, g * BC:(g + 1) * BC, :] = oe.reshape(
            2, 128, 2, BC).transpose(0, 3, 2, 1).reshape(2, BC, PROJ)
    # fw end0 = t=0, end1 = t=T-1;  bw (reversed) end0 = t=T-1, end1 = t=0
    out0 = np.concatenate([h[0, 0], h[1, 1]], axis=1)
    outT = np.concatenate([h[0, 1], h[1, 0]], axis=1)
    emb = (out0 + outT) / np.float32(2.0)
    ss = np.maximum(np.sum(emb * emb, axis=-1, keepdims=True),
                    np.float32(1e-12))
    return (emb / np.sqrt(ss)).astype(np.float32)


def kernel(**inputs):
    from concourse.bass_utils import run_bass_kernel_spmd

    batch = np.asarray(inputs["batch"])
    assert batch.shape == (T, B, F), batch.shape
    in_maps, use_bias = make_in_maps(inputs)
    nc = _get_nc(use_bias, T)
    res = run_bass_kernel_spmd(nc, in_maps, core_ids=list(range(NCORES)))
    return assemble(res.results)


# revision 19
# speedup vs baseline: 1.0272x; 1.0272x over previous
"""Trainium2 Bass kernel for a 3-layer bidirectional projected-LSTM embedder.

Model (from the reference):
  T=160, B=640, F=40, HID=768, PROJ=256, 3 stacked LSTM-with-projection
  layers per direction (fw, bw).  Per step:
      z = [x_t, h_{t-1}] @ Wk + b            # [B, 4*HID], gate order i,j,f,o
      c = sig(f+1)*c + sig(i)*tanh(j)
      h = (sig(o)*tanh(c)) @ Wp              # [B, PROJ]
  Output = l2norm((concat(fw,bw)[t=0] + concat(fw,bw)[t=T-1]) / 2)  # [B, 512]

Sharding: 4 batch groups x 2 directions = 8 cores, BC=160 per core, one
direction per core.  The bw cores receive the input time-reversed host-side,
so every core runs the SAME program (pure forward scan); the host maps the
two saved end-states back to true time.

Layout: feature-major.  All on-chip tensors are [feature(128-partitions),
batch(160)] tiles; z^T = Wk^T @ [x;h]^T is computed with the bf16 weights as
the 128x128 stationary operand (fast-weight-load keeps LDWEIGHTS hidden
under the 160-column stream) and the bf16 activations streaming (the walrus
verifier requires matching dtypes when either operand is fp32).  Gates come
out gate-major, so gate activations, the c update, and s = sig(o)*tanh(c)
need no transposes, and h^T = Wp^T @ s^T is again feature-major -- the
recurrence closes with zero per-step transposes.  The layer-0 x operand is
zero-padded from K=40 to K=128: matmul cost depends on columns, not K, and
a K<128 stationary in the stream both breaks LDWEIGHTS pipelining and drops
PE array duty enough to trip the HAM clock-throttle every super-step.

The three layers run as a pipelined wavefront (super-step n runs layer l at
step n-l), so each layer's ACT/DVE gate chain hides under the other layers'
matmuls.  Wk column order is permuted host-side to [j|i|f|o] so each gate is
one contiguous 2-PSUM-bank block consumed by a single ACT instruction.
Layer-to-layer h stays in SBUF (no DRAM round-trips).  The final
(t0+tT)/2 + l2-normalize is done on the host in numpy.
"""

import numpy as np

T, B, F = 160, 640, 40
HID, PROJ = 768, 256
NG = 4 * HID          # 3072
NCORES = 8
NGRP = 4              # batch groups
BC = B // NGRP        # 160 per core
NT = NG // 128        # 24 gate n-tiles
GPT = 6               # n-tiles per gate
XCH = 16              # x-input DMA chunk (time steps)

_BUILD_CACHE = {}
DEBUG_DUMP = False

# Wk column permutation: gate order i,j,f,o -> [j | i | f | o]
_WK_PERM = np.concatenate([np.arange(768, 1536), np.arange(0, 768),
                           np.arange(1536, 2304), np.arange(2304, 3072)])


def _build(use_bias, t_steps):
    from contextlib import ExitStack

    import concourse.bass as bass  # noqa: F401
    import concourse.tile as tile
    from concourse import bacc, mybir

    f32 = mybir.dt.float32
    f32r = mybir.dt.float32r
    bf16 = mybir.dt.bfloat16
    AF = mybir.ActivationFunctionType

    nc = bacc.Bacc(None, target_bir_lowering=False)

    xT = nc.declare_dram_parameter("xT", [F, t_steps * BC], bf16, isOutput=False)
    wk_in = []
    wp_in = []
    bs_in = []
    for l in range(3):
        ind = F if l == 0 else PROJ
        wk_in.append(nc.declare_dram_parameter(
            f"Wk{l}", [ind + PROJ, NG], bf16, isOutput=False))
        wp_in.append(nc.declare_dram_parameter(
            f"Wp{l}", [HID, PROJ], bf16, isOutput=False))
        if use_bias:
            bs_in.append(nc.declare_dram_parameter(
                f"bs{l}", [128, NT], f32, isOutput=False))
    # h^T of the top layer after the first and last step: [end, 128, m*BC]
    out_ends = nc.declare_dram_parameter(
        "out_ends", [2, 128, 2 * BC], f32r, isOutput=True)
    if DEBUG_DUMP:
        dbg = nc.declare_dram_parameter(
            "dbg", [10, 128, GPT * BC], f32, isOutput=True)

    # k-tile row spans of Wk per layer: x-part rows then h-part rows
    def kspans(l):
        if l == 0:
            return [(0, F)], [(F, 128), (F + 128, 128)]
        return [(0, 128), (128, 128)], [(256, 128), (384, 128)]

    with tile.TileContext(nc) as tc:
        with ExitStack() as top:
            wpool = top.enter_context(tc.tile_pool(name="w", bufs=1))
            st = top.enter_context(tc.tile_pool(name="st", bufs=1))
            hpool = top.enter_context(tc.tile_pool(name="h", bufs=2))
            xpool = top.enter_context(tc.tile_pool(name="x", bufs=2))
            zpool = top.enter_context(
                tc.tile_pool(name="z", bufs=3, space="PSUM"))
            hps = top.enter_context(
                tc.tile_pool(name="hp", bufs=2, space="PSUM"))

            # ---- x-input buffers first: the first matmul needs the x
            # chunk, so its DMA must not queue behind all the weight DMAs
            h_cur = [None, None, None]   # most recent h^T tile   [128, 2*BC]
            xin = [None, None]           # x chunk double buffer
            xbufs = []                   # pre-zeroed [128, .] chunk buffers
            # x is padded to K=128 with zero rows 40:128 so the layer-0
            # x-part matmul is a normal full-K MM (K=40 stalls the LDW
            # pipeline and trips the HAM throttle every super-step).
            for b in range(2):
                xb = xpool.tile([128, XCH * BC], bf16, name=f"xin{b}",
                                tag=f"xin{b}")
                nc.vector.memset(xb, 0)
                xbufs.append(xb)

            def load_xchunk(ch):
                nch = (t_steps + XCH - 1) // XCH
                if ch >= nch:
                    return
                n = min(XCH, t_steps - ch * XCH)
                xt_sb = xbufs[ch % 2]
                nc.sync.dma_start(
                    out=xt_sb[0:F, 0:n * BC],
                    in_=xT[:, ch * XCH * BC:(ch * XCH + n) * BC])
                xin[ch % 2] = xt_sb

            load_xchunk(0)

            # ---- weights to SBUF (layer order: a layer's tiles are only
            # needed once the wavefront reaches it) ----
            wkx, wkh, wpt, bst = [], [], [], []
            for l in range(3):
                xs, hs = kspans(l)
                xt, ht = [], []
                for ki, (r0, rc) in enumerate(xs + hs):
                    pad = 128 if rc < 128 else rc
                    wt = wpool.tile([pad, NG], bf16, name=f"wk{l}_{ki}",
                                    tag=f"wk{l}_{ki}")
                    if pad != rc:
                        nc.vector.memset(wt, 0)
                    for c in range(6):
                        nc.sync.dma_start(
                            out=wt[0:rc, c * 512:(c + 1) * 512],
                            in_=wk_in[l][r0:r0 + rc, c * 512:(c + 1) * 512])
                    (xt if ki < len(xs) else ht).append(wt)
                wkx.append(xt)
                wkh.append(ht)
                pt = []
                for ki in range(6):
                    w = wpool.tile([128, PROJ], bf16, name=f"wp{l}_{ki}",
                                   tag=f"wp{l}_{ki}")
                    nc.sync.dma_start(
                        out=w, in_=wp_in[l][ki * 128:(ki + 1) * 128, :])
                    pt.append(w)
                wpt.append(pt)
                if use_bias:
                    bt = wpool.tile([128, NT], f32, name=f"bs{l}", tag=f"bs{l}")
                    nc.sync.dma_start(out=bt, in_=bs_in[l])
                    bst.append(bt)

            # ---- persistent per-layer state ----
            c_sb = [st.tile([128, GPT * BC], f32, name=f"c{l}", tag=f"c{l}")
                    for l in range(3)]
            gsb = [[st.tile([128, GPT * BC], f32, name=f"g{l}_{g}",
                            tag=f"g{l}_{g}") for g in range(4)]
                   for l in range(3)]
            tc_sb = [st.tile([128, GPT * BC], f32, name=f"tc{l}", tag=f"tc{l}")
                     for l in range(3)]
            tmp_sb = [st.tile([128, GPT * BC], f32, name=f"tm{l}", tag=f"tm{l}")
                      for l in range(3)]
            s_sb = [st.tile([128, GPT * BC], bf16, name=f"s{l}", tag=f"s{l}")
                    for l in range(3)]


            def emit_z_act_dve(l, s):
                # gather rhs k-tiles (f32r activations) + matching weights
                if l == 0:
                    if s % XCH == 0:
                        load_xchunk(s // XCH + 1)
                    xt_sb = xin[(s // XCH) % 2]
                    off = (s % XCH) * BC
                    rhs = [xt_sb[:, off:off + BC]]
                else:
                    hb = h_cur[l - 1]
                    rhs = [hb[:, 0:BC], hb[:, BC:2 * BC]]
                wts = list(wkx[l])
                if s > 0:
                    ho = h_cur[l]
                    rhs += [ho[:, 0:BC], ho[:, BC:2 * BC]]
                    wts += wkh[l]
                nk = len(rhs)

                # z matmuls + gate activation, one gate (6 n-tiles) at a time
                for g in range(4):
                    zg = zpool.tile([128, 1024], f32, name=f"z{l}", tag="zg")
                    for n6 in range(GPT):
                        nt = g * GPT + n6
                        dst = zg[:, (n6 // 3) * 512 + (n6 % 3) * BC:]
                        dst = dst[:, 0:BC]
                        for ki in range(nk):
                            # start=True bulk-clears the whole PSUM bank's
                            # has_written bits, racing with the previous
                            # group's drain -- only the first group per bank
                            # may clear; later groups rely on per-element
                            # overwrite-where-unset.
                            nc.tensor.matmul(
                                dst, wts[ki][:, nt * 128:(nt + 1) * 128],
                                rhs[ki],
                                start=(ki == 0 and n6 % 3 == 0),
                                stop=(ki == nk - 1),
                                skip_group_check=(n6 % 3 != 0))
                    fn = AF.Tanh if g == 0 else AF.Sigmoid
                    gd = gsb[l][g]
                    if use_bias:
                        for n6 in range(GPT):
                            nt = g * GPT + n6
                            src = zg[:, (n6 // 3) * 512 + (n6 % 3) * BC:]
                            nc.scalar.activation(
                                gd[:, n6 * BC:(n6 + 1) * BC], src[:, 0:BC],
                                fn, bias=bst[l][:, nt:nt + 1],
                                scale=1.0)
                    else:
                        bias = 1.0 if g == 2 else 0.0
                        src = zg.rearrange("p (b c) -> p b c", b=2)[:, :, 0:3 * BC]
                        dstv = gd.rearrange("p (b c) -> p b c", b=2)
                        nc.scalar.activation(dstv, src, fn, bias=bias)
                    if DEBUG_DUMP and g == 0 and (l, s) in ((0, 0), (0, 1), (1, 0), (1, 1), (2, 0)):
                        slot = {(0, 0): 0, (0, 1): 1, (1, 0): 2,
                                (1, 1): 3, (2, 0): 4}[(l, s)]
                        zt = st.tile([128, GPT * BC], f32, name=f"dbz{slot}",
                                     tag=f"dbz{slot}")
                        nc.vector.tensor_copy(
                            zt.rearrange("p (b c) -> p b c", b=2),
                            zg.rearrange("p (b c) -> p b c", b=2)[:, :, 0:3 * BC])
                        nc.sync.dma_start(out=dbg[slot], in_=zt)

                # c = sig(f+1)*c + sig(i)*tanh(j);  s = sig(o)*tanh(c)
                gj, gi, gf, go = gsb[l]
                if s == 0:
                    nc.vector.tensor_mul(c_sb[l], gi, gj)
                else:
                    nc.vector.tensor_mul(tmp_sb[l], gi, gj)
                    nc.vector.tensor_mul(c_sb[l], gf, c_sb[l])
                    nc.vector.tensor_add(c_sb[l], c_sb[l], tmp_sb[l])
                nc.scalar.activation(tc_sb[l], c_sb[l], AF.Tanh)
                nc.vector.tensor_mul(s_sb[l], go, tc_sb[l])
                if DEBUG_DUMP and (l, s) in ((0, 0), (1, 0)):
                    slot = 8 if l == 0 else 9
                    st9 = st.tile([128, GPT * BC], f32, name=f"dbs{slot}",
                                  tag=f"dbs{slot}")
                    nc.vector.tensor_copy(st9, s_sb[l])
                    nc.sync.dma_start(out=dbg[slot], in_=st9)

            def emit_wp(l, s):
                hp = hps.tile([128, 2 * BC], f32, name="hps", tag="hps")
                for m in range(2):
                    for ki in range(6):
                        nc.tensor.matmul(
                            hp[:, m * BC:(m + 1) * BC],
                            wpt[l][ki][:, m * 128:(m + 1) * 128],
                            s_sb[l][:, ki * BC:(ki + 1) * BC],
                            start=(ki == 0 and m == 0), stop=(ki == 5),
                            skip_group_check=(m == 1))
                hn = hpool.tile([128, 2 * BC], bf16, name=f"hn{l}", tag=f"hn{l}")
                nc.vector.tensor_copy(hn, hp)
                h_cur[l] = hn
                if DEBUG_DUMP and (l, s) in ((0, 0), (1, 0), (0, 1)):
                    slot = {(0, 0): 5, (1, 0): 6, (0, 1): 7}[(l, s)]
                    ht = st.tile([128, 2 * BC], f32, name=f"dbh{slot}",
                                 tag=f"dbh{slot}")
                    nc.vector.tensor_copy(ht, hp)
                    nc.sync.dma_start(out=dbg[slot][:, 0:2 * BC], in_=ht)
                if l == 2 and (s == 0 or s == t_steps - 1):
                    end = 0 if s == 0 else 1
                    oc = st.tile([128, 2 * BC], f32r, name=f"oc{end}",
                                 tag=f"oc{end}")
                    nc.vector.tensor_copy(oc, hp)
                    nc.sync.dma_start(out=out_ends[end], in_=oc)

            # ---- wavefront: super-step n runs layer l at step n-l; each
            # layer's Wp matmul is emitted one z-block later so the PE never
            # waits on the ACT/DVE gate chain.
            wp_queue = []
            for ss in range(t_steps + 2):
                for l in range(3):
                    s = ss - l
                    if not (0 <= s < t_steps):
                        continue
                    # flush any deferred Wp this z-block depends on
                    need = {(l, s - 1), (l - 1, s)}
                    while any(x in wp_queue for x in need):
                        emit_wp(*wp_queue.pop(0))
                    emit_z_act_dve(l, s)
                    wp_queue.append((l, s))
                    if len(wp_queue) > 1:
                        emit_wp(*wp_queue.pop(0))
            while wp_queue:
                emit_wp(*wp_queue.pop(0))

    nc.finalize()
    return nc


def _get_nc(use_bias, t_steps):
    key = (use_bias, t_steps)
    if key not in _BUILD_CACHE:
        _BUILD_CACHE[key] = _build(use_bias, t_steps)
    return _BUILD_CACHE[key]


def make_in_maps(inputs, t_steps=T):
    """Per-core input dicts.  Cores 0-3: fw, batch groups 0-3.
    Cores 4-7: bw (time-reversed input), batch groups 0-3."""
    import ml_dtypes

    inp = {k: np.asarray(v, dtype=np.float32) for k, v in inputs.items()}
    batch = inp["batch"][:t_steps]

    shared = {}
    for d in ("fw", "bw"):
        for l in range(3):
            shared[d, f"Wk{l}"] = np.ascontiguousarray(
                inp[f"Wk_{d}{l}"][:, _WK_PERM].astype(ml_dtypes.bfloat16))
            shared[d, f"Wp{l}"] = np.ascontiguousarray(
                inp[f"Wp_{d}{l}"].astype(ml_dtypes.bfloat16))
            b = inp[f"b_{d}{l}"][_WK_PERM]
            shared[d, f"bs{l}"] = np.ascontiguousarray(
                b.reshape(NT, 128).T.astype(np.float32))

    use_bias = any(np.any(inp[f"b_{d}{l}"])
                   for d in ("fw", "bw") for l in range(3))
    in_maps = []
    for i in range(NCORES):
        d = "fw" if i < NGRP else "bw"
        g = i % NGRP
        xb = batch[:, g * BC:(g + 1) * BC, :]            # [T, BC, F]
        if d == "bw":
            xb = xb[::-1]
        xT_i = np.ascontiguousarray(
            xb.transpose(2, 0, 1).reshape(F, t_steps * BC)
            .astype(ml_dtypes.bfloat16))
        m = {"xT": xT_i}
        for l in range(3):
            m[f"Wk{l}"] = shared[d, f"Wk{l}"]
            m[f"Wp{l}"] = shared[d, f"Wp{l}"]
            if use_bias:
                m[f"bs{l}"] = shared[d, f"bs{l}"]
        in_maps.append(m)
    return in_maps, use_bias


def assemble(results, t_steps=T):
    """results[i]["out_ends"]: [end, m, 128, BC] -> final [B, 2*PROJ] f32."""
    h = np.zeros((2, 2, B, PROJ), dtype=np.float32)   # [dir, end, B, PROJ]
    for i in range(NCORES):
        di, g = (0, i) if i < NGRP else (1, i - NGRP)
        oe = np.asarray(results[i]["out_ends"], dtype=np.float32)
        # oe: [end, p, m*BC];  h[b, m*128 + p] = oe[end, p, m*BC + b]
        h[di, :, g * BC:(g + 1) * BC, :] = oe.reshape(
            2, 128, 2, BC).transpose(0, 3, 2, 1).reshape(2, BC, PROJ)
    # fw end0 = t=0, end1 = t=T-1;  bw (reversed) end0 = t=T-1, end1 = t=0
    out0 = np.concatenate([h[0, 0], h[1, 1]], axis=1)
    outT = np.concatenate([h[0, 1], h[1, 0]], axis=1)
    emb = (out0 + outT) / np.float32(2.0)
    ss = np.maximum(np.sum(emb * emb, axis=-1, keepdims=True),
                    np.float32(1e-12))
    return (emb / np.sqrt(ss)).astype(np.float32)


def kernel(**inputs):
    from concourse.bass_utils import run_bass_kernel_spmd

    batch = np.asarray(inputs["batch"])
    assert batch.shape == (T, B, F), batch.shape
    in_maps, use_bias = make_in_maps(inputs)
    nc = _get_nc(use_bias, T)
    res = run_bass_kernel_spmd(nc, in_maps, core_ids=list(range(NCORES)))
    return assemble(res.results)
